# revision 36
# baseline (speedup 1.0000x reference)
"""GCN layer on 8 Trainium2 NeuronCores.

out = D^-1/2 A D^-1/2 (values @ W + b),  A: [8192, 8192] f32 dense.

Strategy (row-parallel, single streaming pass, no collectives):
- Host folds the symmetric normalization into the operands: d = A @ 1 and
  dis = 1/(sqrt(d)+eps) are tiny host-side reductions, then
  vts = (values * dis_j)^T and at_k = (A[rows_k] * dis_i)^T are cast to fp16.
- at_k is laid out partition-major ([128, jt, i]) so every DMA partition
  line is an 8KB contiguous DRAM read.
- Core k streams its 16MB fp16 at_k slab once, accumulating
  outT[o, i] += fcY[jt]^T @ at[jt] in PSUM across all 64 j-tiles while the
  DMA stream is still in flight; fcY = vts^T @ W is computed on-device at
  the start (replicated, cheaper than an all-gather).
- b (zero in this problem) is handled host-side via its rank-1
  contribution dis_i * (A @ dis) * b^T when nonzero.
"""
import os
import numpy as np

N, D, OUT = 8192, 128, 128
N_CORES = 8
ROWS = N // N_CORES          # 1024 rows of A per core
NJT = N // 128               # 64 j-tiles
# jt 0..PRE_JT-1 are prefetched into a resident tile right after vts, so
# the accumulation's start group never waits on the stream and the PE
# pipeline warms early; the streamed chunks cover jt PRE_JT..63 with a
# short taper so the stop matmul depends on one sharp 256KB arrival
# instead of a trickle of latency-bound small DMAs
PRE_JT = 4
CHUNKS = [4] * 15  # sums to 60 = 64 - PRE_JT

_CACHE = {}


def _build():
    import concourse.bacc as bacc
    import concourse.mybir as mybir
    import concourse.tile as tile

    F32, F16 = mybir.dt.float32, mybir.dt.float16
    nc = bacc.Bacc(None, target_bir_lowering=False, num_devices=N_CORES)

    # at[p, jt*ROWS + i] = A[i0+i, jt*128+p] * dis[i0+i]  (fp16)
    at_in = nc.declare_dram_parameter("at", [128, NJT * ROWS], F16, isOutput=False)
    vts_in = nc.declare_dram_parameter("vts", [D, N], F16, isOutput=False)
    w_in = nc.declare_dram_parameter("w", [D, OUT], F16, isOutput=False)
    outT = nc.declare_dram_parameter("outT", [OUT, ROWS], F16, isOutput=True)

    with tile.TileContext(nc) as tc:
        with (
            tc.tile_pool(name="const", bufs=1) as constp,
            tc.tile_pool(name="stage", bufs=6) as stage,
            tc.tile_pool(name="psfc", bufs=2, space="PSUM") as psfc,
            tc.tile_pool(name="psacc", bufs=2, space="PSUM") as psacc,
        ):
            # vts/w head both queues (small, gate the fc prologue), split
            # evenly so neither queue starts the A stream with a byte lead;
            # A chunks then alternate whole-chunk between the two queues
            # (keeps 8KB-contiguous partition lines per DMA)
            w_sb = constp.tile([D, OUT], F16)
            nc.scalar.dma_start(out=w_sb[:], in_=w_in[:])
            vts_sb = constp.tile([D, N], F16)
            for q in range(4):
                eng = nc.sync if q % 2 == 0 else nc.scalar
                eng.dma_start(
                    out=vts_sb[:, q * 2048 : (q + 1) * 2048],
                    in_=vts_in[:, q * 2048 : (q + 1) * 2048],
                )

            # prefetch jt 0..PRE_JT-1 (resident for the whole kernel)
            pre = constp.tile([128, PRE_JT * ROWS], F16)
            ph = PRE_JT * ROWS // 2
            nc.sync.dma_start(out=pre[:, :ph], in_=at_in[:, :ph])
            nc.scalar.dma_start(out=pre[:, ph:], in_=at_in[:, ph : PRE_JT * ROWS])

            # every chunk is half-split across both queues so they advance
            # in lockstep — a whole-chunk alternation lets one queue lag
            # and stalls PE (measured: +7us)
            st_tiles = [(pre, 0, PRE_JT)]
            off = PRE_JT
            for c, tch in enumerate(CHUNKS):
                st = stage.tile([128, tch * ROWS], F16, tag=f"st{tch}")
                half = tch * ROWS // 2
                nc.sync.dma_start(
                    out=st[:, :half],
                    in_=at_in[:, off * ROWS : off * ROWS + half],
                )
                nc.scalar.dma_start(
                    out=st[:, half:],
                    in_=at_in[:, off * ROWS + half : (off + tch) * ROWS],
                )
                st_tiles.append((st, off, tch))
                off += tch

            # fcY[p, nt*128 + o] = dis_j * fc[nt*128+p, o], j = nt*128+p.
            # The fc chain drains at ~2.6us/group (PE -> DVE cast -> sem ->
            # PE through the 2-bank psfc ring) and PE is in-order, so pace
            # emission at one group per chunk — matching the chain latency —
            # staying 2 chunks ahead of each group's first consumer. Bulk
            # emission (before or inside the stream) stalls PE; lazy
            # emission gates the final chunks (both measured slower).
            fcY = constp.tile([128, N], F16)
            fc_done = 0

            def emit_fc_through(n_groups):
                nonlocal fc_done
                while fc_done < min(NJT // 4, n_groups):
                    g = fc_done
                    ps = psfc.tile([128, 512], F32, tag="fc")
                    for m in range(4):
                        # one accumulation group per PSUM tile: only the
                        # first write clears the bank's has_written bits
                        nt = g * 4 + m
                        nc.tensor.matmul(
                            ps[:, m * 128 : (m + 1) * 128],
                            vts_sb[:, nt * 128 : (nt + 1) * 128], w_sb[:],
                            start=(m == 0), stop=(m == 3),
                        )
                    nc.vector.tensor_copy(fcY[:, g * 512 : (g + 1) * 512], ps[:])
                    fc_done += 1

            # main stream: outT[o, i] += sum_jt fcY[jt]^T @ at[jt]
            oT = [
                psacc.tile([128, 512], F32, tag="acc", name=f"oT{h}")
                for h in range(2)
            ]
            for c, (st, off, tch) in enumerate(st_tiles):
                # lazy demand-driven pace (1 group per 4-jt chunk, matching
                # the chain latency) plus a slow ramp ahead so the last
                # groups finish a few chunks before their consumers
                emit_fc_through((off + tch + 3) // 4 + min(2, c // 7))
                for m in range(tch):
                    jt = off + m
                    for h in range(2):
                        nc.tensor.matmul(
                            oT[h][:],
                            fcY[:, jt * 128 : (jt + 1) * 128],
                            st[:, m * ROWS + h * 512 : m * ROWS + (h + 1) * 512],
                            start=(jt == 0), stop=(jt == NJT - 1),
                        )

            for h in range(2):
                osb = stage.tile([128, 512], F16, tag="osb")
                nc.vector.tensor_copy(osb[:], oT[h][:])
                nc.scalar.dma_start(out=outT[:, h * 512 : (h + 1) * 512], in_=osb[:])

    nc.compile()
    return nc


def kernel(values, adjacency, W, b):
    from concourse.bass_utils import run_bass_kernel_spmd

    if "nc" not in _CACHE:
        _CACHE["nc"] = _build()
    nc = _CACHE["nc"]

    values = np.asarray(values, dtype=np.float32)
    adjacency = np.asarray(adjacency, dtype=np.float32)
    W = np.asarray(W, dtype=np.float32)
    b = np.asarray(b, dtype=np.float32)

    d = adjacency.sum(axis=1, dtype=np.float32)
    dis = (1.0 / (np.sqrt(d) + 1e-8)).astype(np.float32)   # [N]

    vts = np.ascontiguousarray((values * dis[:, None]).T).astype(np.float16)
    w16 = W.astype(np.float16)

    in_maps = []
    for k in range(N_CORES):
        sl = slice(k * ROWS, (k + 1) * ROWS)
        a_sc = (adjacency[sl] * dis[sl][:, None]).astype(np.float16)
        # [i, jt, p] -> [p, jt, i], each partition line contiguous per chunk
        at = np.ascontiguousarray(
            a_sc.reshape(ROWS, NJT, 128).transpose(2, 1, 0)
        ).reshape(128, NJT * ROWS)
        in_maps.append({"at": at, "vts": vts, "w": w16})

    trace = bool(int(os.environ.get("GCN_TRACE", "0")))
    res = run_bass_kernel_spmd(nc, in_maps, list(range(N_CORES)), trace=trace)
    if trace and res.exec_time_ns is not None:
        print(f"HW exec time: {res.exec_time_ns} ns")
        _CACHE["exec_time_ns"] = res.exec_time_ns

    out = np.concatenate(
        [res.results[k]["outT"].T for k in range(N_CORES)], axis=0
    ).astype(np.float32)
    if np.any(b):
        s = adjacency @ dis
        out += (dis * s)[:, None] * b[None, :]
    return out


# revision 37
# speedup vs baseline: 1.0291x; 1.0291x over previous
"""GCN layer on 8 Trainium2 NeuronCores.

out = D^-1/2 A D^-1/2 (values @ W + b),  A: [8192, 8192] f32 dense.

Strategy (row-parallel, single streaming pass, no collectives):
- Host folds the symmetric normalization into the operands: d = A @ 1 and
  dis = 1/(sqrt(d)+eps) are tiny host-side reductions, then
  vts = (values * dis_j)^T and at_k = (A[rows_k] * dis_i)^T are cast to fp16.
- at_k is laid out partition-major ([128, jt, i]) so every DMA partition
  line is an 8KB contiguous DRAM read.
- Core k streams its 16MB fp16 at_k slab once, accumulating
  outT[o, i] += fcY[jt]^T @ at[jt] in PSUM across all 64 j-tiles while the
  DMA stream is still in flight; fcY = vts^T @ W is computed on-device at
  the start (replicated, cheaper than an all-gather).
- b (zero in this problem) is handled host-side via its rank-1
  contribution dis_i * (A @ dis) * b^T when nonzero.
"""
import os
import numpy as np

N, D, OUT = 8192, 128, 128
N_CORES = 8
ROWS = N // N_CORES          # 1024 rows of A per core
NJT = N // 128               # 64 j-tiles
# jt 0..PRE_JT-1 are prefetched into a resident tile right after vts, so
# the accumulation's start group never waits on the stream and the PE
# pipeline warms early; the streamed chunks cover jt PRE_JT..63 with a
# short taper so the stop matmul depends on one sharp 256KB arrival
# instead of a trickle of latency-bound small DMAs
PRE_JT = 4
CHUNKS = [4] * 14 + [2, 1, 1]  # sums to 60 = 64 - PRE_JT

_CACHE = {}


def _build():
    import concourse.bacc as bacc
    import concourse.mybir as mybir
    import concourse.tile as tile

    F32, F16 = mybir.dt.float32, mybir.dt.float16
    nc = bacc.Bacc(None, target_bir_lowering=False, num_devices=N_CORES)

    # at[p, jt*ROWS + i] = A[i0+i, jt*128+p] * dis[i0+i]  (fp16)
    at_in = nc.declare_dram_parameter("at", [128, NJT * ROWS], F16, isOutput=False)
    vts_in = nc.declare_dram_parameter("vts", [D, N], F16, isOutput=False)
    w_in = nc.declare_dram_parameter("w", [D, OUT], F16, isOutput=False)
    outT = nc.declare_dram_parameter("outT", [OUT, ROWS], F16, isOutput=True)

    with tile.TileContext(nc) as tc:
        with (
            tc.tile_pool(name="const", bufs=1) as constp,
            tc.tile_pool(name="stage", bufs=6) as stage,
            tc.tile_pool(name="psfc", bufs=2, space="PSUM") as psfc,
            tc.tile_pool(name="psacc", bufs=2, space="PSUM") as psacc,
        ):
            # vts/w head both queues (small, gate the fc prologue), split
            # evenly so neither queue starts the A stream with a byte lead;
            # A chunks then alternate whole-chunk between the two queues
            # (keeps 8KB-contiguous partition lines per DMA)
            w_sb = constp.tile([D, OUT], F16)
            nc.scalar.dma_start(out=w_sb[:], in_=w_in[:])
            vts_sb = constp.tile([D, N], F16)
            for q in range(4):
                eng = nc.sync if q % 2 == 0 else nc.scalar
                eng.dma_start(
                    out=vts_sb[:, q * 2048 : (q + 1) * 2048],
                    in_=vts_in[:, q * 2048 : (q + 1) * 2048],
                )

            # prefetch jt 0..PRE_JT-1 (resident for the whole kernel)
            pre = constp.tile([128, PRE_JT * ROWS], F16)
            ph = PRE_JT * ROWS // 2
            nc.sync.dma_start(out=pre[:, :ph], in_=at_in[:, :ph])
            nc.scalar.dma_start(out=pre[:, ph:], in_=at_in[:, ph : PRE_JT * ROWS])

            # every chunk is half-split across both queues so they advance
            # in lockstep — a whole-chunk alternation lets one queue lag
            # and stalls PE (measured: +7us)
            st_tiles = [(pre, 0, PRE_JT)]
            off = PRE_JT
            for c, tch in enumerate(CHUNKS):
                st = stage.tile([128, tch * ROWS], F16, tag=f"st{tch}")
                half = tch * ROWS // 2
                nc.sync.dma_start(
                    out=st[:, :half],
                    in_=at_in[:, off * ROWS : off * ROWS + half],
                )
                nc.scalar.dma_start(
                    out=st[:, half:],
                    in_=at_in[:, off * ROWS + half : (off + tch) * ROWS],
                )
                st_tiles.append((st, off, tch))
                off += tch

            # fcY[p, nt*128 + o] = dis_j * fc[nt*128+p, o], j = nt*128+p.
            # The fc chain drains at ~2.6us/group (PE -> DVE cast -> sem ->
            # PE through the 2-bank psfc ring) and PE is in-order, so pace
            # emission at one group per chunk — matching the chain latency —
            # staying 2 chunks ahead of each group's first consumer. Bulk
            # emission (before or inside the stream) stalls PE; lazy
            # emission gates the final chunks (both measured slower).
            fcY = constp.tile([128, N], F16)
            fc_done = 0

            def emit_fc_through(n_groups):
                nonlocal fc_done
                while fc_done < min(NJT // 4, n_groups):
                    g = fc_done
                    ps = psfc.tile([128, 512], F32, tag="fc")
                    for m in range(4):
                        # one accumulation group per PSUM tile: only the
                        # first write clears the bank's has_written bits
                        nt = g * 4 + m
                        nc.tensor.matmul(
                            ps[:, m * 128 : (m + 1) * 128],
                            vts_sb[:, nt * 128 : (nt + 1) * 128], w_sb[:],
                            start=(m == 0), stop=(m == 3),
                        )
                    nc.vector.tensor_copy(fcY[:, g * 512 : (g + 1) * 512], ps[:])
                    fc_done += 1

            # main stream: outT[o, i] += sum_jt fcY[jt]^T @ at[jt]
            oT = [
                psacc.tile([128, 512], F32, tag="acc", name=f"oT{h}")
                for h in range(2)
            ]
            for c, (st, off, tch) in enumerate(st_tiles):
                # lazy demand-driven pace (1 group per 4-jt chunk, matching
                # the chain latency) plus a slow ramp ahead so the last
                # groups finish a few chunks before their consumers
                emit_fc_through((off + tch + 3) // 4 + min(2, c // 7))
                for m in range(tch):
                    jt = off + m
                    for h in range(2):
                        nc.tensor.matmul(
                            oT[h][:],
                            fcY[:, jt * 128 : (jt + 1) * 128],
                            st[:, m * ROWS + h * 512 : m * ROWS + (h + 1) * 512],
                            start=(jt == 0), stop=(jt == NJT - 1),
                        )

            for h in range(2):
                osb = stage.tile([128, 512], F16, tag="osb")
                nc.vector.tensor_copy(osb[:], oT[h][:])
                nc.scalar.dma_start(out=outT[:, h * 512 : (h + 1) * 512], in_=osb[:])

    nc.compile()
    return nc


def kernel(values, adjacency, W, b):
    from concourse.bass_utils import run_bass_kernel_spmd

    if "nc" not in _CACHE:
        _CACHE["nc"] = _build()
    nc = _CACHE["nc"]

    values = np.asarray(values, dtype=np.float32)
    adjacency = np.asarray(adjacency, dtype=np.float32)
    W = np.asarray(W, dtype=np.float32)
    b = np.asarray(b, dtype=np.float32)

    d = adjacency.sum(axis=1, dtype=np.float32)
    dis = (1.0 / (np.sqrt(d) + 1e-8)).astype(np.float32)   # [N]

    vts = np.ascontiguousarray((values * dis[:, None]).T).astype(np.float16)
    w16 = W.astype(np.float16)

    in_maps = []
    for k in range(N_CORES):
        sl = slice(k * ROWS, (k + 1) * ROWS)
        a_sc = (adjacency[sl] * dis[sl][:, None]).astype(np.float16)
        # [i, jt, p] -> [p, jt, i], each partition line contiguous per chunk
        at = np.ascontiguousarray(
            a_sc.reshape(ROWS, NJT, 128).transpose(2, 1, 0)
        ).reshape(128, NJT * ROWS)
        in_maps.append({"at": at, "vts": vts, "w": w16})

    trace = bool(int(os.environ.get("GCN_TRACE", "0")))
    res = run_bass_kernel_spmd(nc, in_maps, list(range(N_CORES)), trace=trace)
    if trace and res.exec_time_ns is not None:
        print(f"HW exec time: {res.exec_time_ns} ns")
        _CACHE["exec_time_ns"] = res.exec_time_ns

    out = np.concatenate(
        [res.results[k]["outT"].T for k in range(N_CORES)], axis=0
    ).astype(np.float32)
    if np.any(b):
        s = adjacency @ dis
        out += (dis * s)[:, None] * b[None, :]
    return out
